# revision 1
# baseline (speedup 1.0000x reference)
"""HBitLinear Trainium2 kernel.

out = quant4(x @ H_1024) @ ternary(W).T, x:[8,8192,1024] f32, W:[1024,1024] f32.

Strategy (8 NeuronCores, data-parallel over the batch dim):
  - Each core gets one batch slice x_b [8192,1024]; W is replicated.
  - Hadamard via Kronecker split H_1024 = H_8 (x) H_128 (Sylvester), so
    x@H = FHT8 across 128-column chunks (vector engine butterflies) followed
    by a single 128-contraction matmul against H_128/32 (tensor engine):
    16x fewer PE FLOPs than a dense 1024-contraction matmul.
  - Activation quant: per-token absmax -> scale; round-to-nearest-even via
    the fp32 magic-number trick; quantized ints [-8,7] stored as fp8e4m3
    (exact).  Ternary weights {-1,0,1} also fp8 (exact).  Second matmul runs
    at fp8 rate and its PSUM accumulation is exact integer arithmetic.
  - Epilogue: out = G * scale[token] * wscale[out_feature].
"""

import numpy as np

_CACHE: dict = {}

P = 128          # partitions
ST = 64          # token tiles per core (8192 / 128)
NCHUNK = 8       # 1024 / 128
MAGIC = float(np.float32(3 * 2 ** 22))  # 1.5*2^23: fp32 RNE rounding constant


def _sylvester(k: int) -> np.ndarray:
    h = np.array([[1]], dtype=np.int64)
    for _ in range(k):
        h = np.block([[h, h], [h, -h]])
    return h


def _build():
    import concourse.bass as bass  # noqa: F401
    import concourse.mybir as mybir
    import concourse.tile as tile
    from concourse import bacc

    dt = mybir.dt
    ALU = mybir.AluOpType
    ACTF = mybir.ActivationFunctionType

    nc = bacc.Bacc("TRN2", target_bir_lowering=False, debug=False)

    x = nc.dram_tensor("x", [ST * P, NCHUNK * P], dt.float32, kind="ExternalInput")
    w = nc.dram_tensor("w", [NCHUNK * P, NCHUNK * P], dt.float32, kind="ExternalInput")
    hm = nc.dram_tensor("hm", [P, P], dt.float32, kind="ExternalInput")
    out = nc.dram_tensor("out", [ST * P, NCHUNK * P], dt.float32, kind="ExternalOutput")

    from contextlib import ExitStack

    with tile.TileContext(nc) as tc, ExitStack() as stack:
        # ---------------- persistent constants ----------------
        const = stack.enter_context(tc.tile_pool(name="const", bufs=1))
        hm_sb = const.tile([P, P], dt.float32, tag="hm")
        nc.sync.dma_start(hm_sb[:], hm[:])
        id32 = const.tile([P, P], dt.float32, tag="id32")
        id8 = const.tile([P, P], dt.float8e4, tag="id8")
        id16 = const.tile([P, P], dt.bfloat16, tag="id16")
        from concourse.masks import make_identity
        make_identity(nc, id32[:])
        make_identity(nc, id8[:])
        make_identity(nc, id16[:])
        # ternary weight, transposed: ternT[jc] [j2=128, o=1024] fp8
        ternT = const.tile([P, NCHUNK, P * NCHUNK], dt.float8e4, tag="ternT")
        # broadcast weight scales [128, 1024] fp32
        wsb = const.tile([P, P * NCHUNK], dt.float32, tag="wsb")

        # Main-loop PSUM pools enter FIRST so their bank addresses don't
        # depend on weight-prep frees — lets early token tiles (transpose/
        # butterfly/M1/quant) overlap with the one-time weight prep.
        ps_xT = stack.enter_context(tc.tile_pool(name="ps_xT", bufs=1, space="PSUM"))
        ps_xh = stack.enter_context(tc.tile_pool(name="ps_xh", bufs=2, space="PSUM"))

        # ---------------- weight prep (one-time) ----------------
        ws_dram = nc.dram_tensor("ws_scratch", [NCHUNK * P], dt.float32)
        with tc.tile_pool(name="wprep", bufs=1) as wp, \
             tc.tile_pool(name="wprep_ps", bufs=1, space="PSUM") as wpp:
            w_sb = wp.tile([P, NCHUNK, P * NCHUNK], dt.float32, tag="w")
            nc.sync.dma_start(
                w_sb[:],
                w[:].rearrange("(a p) j -> p a j", p=P),
            )
            ws = wp.tile([P, NCHUNK], dt.float32, tag="ws")
            bpos = wp.tile([P, NCHUNK], dt.float32, tag="bpos")
            bneg = wp.tile([P, NCHUNK], dt.float32, tag="bneg")
            tlt = wp.tile([P, NCHUNK, P * NCHUNK], dt.float32, tag="tlt")
            tern = wp.tile([P, NCHUNK, P * NCHUNK], dt.float8e4, tag="tern")
            for oc in range(NCHUNK):
                # ws = max(mean|w|, 1e-5) per row
                nc.vector.tensor_reduce(
                    ws[:, oc : oc + 1], w_sb[:, oc, :],
                    axis=mybir.AxisListType.X, op=ALU.max if False else ALU.add,
                    apply_absolute_value=True,
                )
                nc.vector.tensor_scalar(
                    ws[:, oc : oc + 1], ws[:, oc : oc + 1],
                    float(np.float32(1.0 / 1024.0)), 1e-5, ALU.mult, ALU.max,
                )
                nc.vector.tensor_scalar_mul(bpos[:, oc : oc + 1], ws[:, oc : oc + 1], 0.5)
                nc.vector.tensor_scalar_mul(bneg[:, oc : oc + 1], ws[:, oc : oc + 1], -0.5)
                # tern = (w > 0.5 ws) - (w < -0.5 ws)  in {-1, 0, 1}
                nc.vector.tensor_scalar(
                    tlt[:, oc, :], w_sb[:, oc, :],
                    bneg[:, oc : oc + 1], None, ALU.is_lt,
                )
                nc.vector.scalar_tensor_tensor(
                    tern[:, oc, :], w_sb[:, oc, :], bpos[:, oc : oc + 1],
                    tlt[:, oc, :], ALU.is_gt, ALU.subtract,
                )
            # transpose tern blocks -> ternT
            for jc in range(NCHUNK):
                tp = wpp.tile([P, NCHUNK, P, 2], dt.float8e4, tag="tp")
                for oc in range(NCHUNK):
                    nc.tensor.transpose(
                        tp[:, oc, :, 0], tern[:, oc, jc * P : (jc + 1) * P], id8[:]
                    )
                nc.scalar.copy(ternT[:, jc, :], tp[:, :, :, 0])
            # wscale broadcast tile: bounce through DRAM, then a partition-
            # stride-0 DMA broadcasts the 1024-vector to all 128 partitions.
            nc.sync.dma_start(
                ws_dram[:].rearrange("(a p) -> p a", p=P), ws[:, :]
            )
            wsb_bcast = bass.AP(
                tensor=ws_dram[:].tensor, offset=0,
                ap=[[0, P]] + list(ws_dram[:].ap),
            )
            nc.gpsimd.dma_start(out=wsb[:, :], in_=wsb_bcast)

        # ---------------- main loop pools ----------------
        xpool = stack.enter_context(tc.tile_pool(name="xin", bufs=4))
        bfly = stack.enter_context(tc.tile_pool(name="bfly", bufs=3))
        qpool = stack.enter_context(tc.tile_pool(name="q", bufs=3))
        opool = stack.enter_context(tc.tile_pool(name="osb", bufs=3))
        scpool = stack.enter_context(tc.tile_pool(name="scales", bufs=4))
        ps_qT = stack.enter_context(tc.tile_pool(name="ps_qT", bufs=1, space="PSUM"))
        ps_g = stack.enter_context(tc.tile_pool(name="ps_g", bufs=1, space="PSUM"))

        for st in range(ST):
            s0 = st * P
            x_t = xpool.tile([P, NCHUNK * P], dt.float32, tag="x")
            nc.sync.dma_start(x_t[:], x[s0 : s0 + P, :])

            # transpose x chunks: xT[c] = x_block(c).T   [i2, s]
            # chunk index c decomposed into bits (g, b, l) = (c>>2, c>>1, c)&1
            # so each butterfly stage is 2 big strided ops instead of 8.
            xT = ps_xT.tile([P, 2, 2, 2, P], dt.float32, tag="xT")
            for c in range(NCHUNK):
                nc.tensor.transpose(
                    xT[:, (c >> 2) & 1, (c >> 1) & 1, c & 1, :],
                    x_t[:, c * P : (c + 1) * P], id32[:],
                )

            # FHT8 butterfly.  Stage 1 mixes one PSUM and one SBUF operand
            # (DVE allows a single PSUM input), so even chunks are first
            # copied to SBUF on the scalar engine.  Stage 2 runs on GpSimd
            # (SBUF-only there) to offload the vector engine.
            v0e = bfly.tile([P, 2, 2, P], dt.float32, tag="v0e")
            nc.scalar.copy(v0e[:, :, :, :], xT[:, :, :, 0, :])
            v1 = bfly.tile([P, 2, 2, 2, P], dt.float32, tag="v1")
            v2 = bfly.tile([P, 2, 2, 2, P], dt.float32, tag="v2")
            v3 = bfly.tile([P, 2, 2, 2, P], dt.float32, tag="v3")
            nc.vector.tensor_add(v1[:, :, :, 0, :], v0e[:], xT[:, :, :, 1, :])
            nc.vector.tensor_sub(v1[:, :, :, 1, :], v0e[:], xT[:, :, :, 1, :])
            nc.gpsimd.tensor_add(v2[:, :, 0, :, :], v1[:, :, 0, :, :], v1[:, :, 1, :, :])
            nc.gpsimd.tensor_sub(v2[:, :, 1, :, :], v1[:, :, 0, :, :], v1[:, :, 1, :, :])
            nc.vector.tensor_add(v3[:, 0, :, :, :], v2[:, 0, :, :, :], v2[:, 1, :, :, :])
            nc.vector.tensor_sub(v3[:, 1, :, :, :], v2[:, 0, :, :, :], v2[:, 1, :, :, :])

            # M1: xh[s, j1*128+j2] = v3[j1].T @ (H128/32)
            xh = ps_xh.tile([P, NCHUNK, P], dt.float32, tag="xh")
            for j1 in range(NCHUNK):
                nc.tensor.matmul(
                    xh[:, j1, :],
                    v3[:, (j1 >> 2) & 1, (j1 >> 1) & 1, j1 & 1, :],
                    hm_sb[:], start=True, stop=True,
                )

            # per-token scale
            amax = scpool.tile([P, 1], dt.float32, tag="amax")
            sc = scpool.tile([P, 1], dt.float32, tag="sc")
            rsc = scpool.tile([P, 1], dt.float32, tag="rsc")
            nc.vector.tensor_reduce(
                amax[:], xh[:, :, :], axis=mybir.AxisListType.XY, op=ALU.max,
                apply_absolute_value=True,
            )
            nc.vector.tensor_scalar(
                sc[:], amax[:], 1e-5, float(np.float32(1.0 / 7.0)), ALU.max, ALU.mult
            )
            nc.vector.reciprocal(rsc[:], sc[:])

            # quantize: q = RNE(xh * rsc) as fp8 (exact ints in [-8,7])
            t_t = qpool.tile([P, NCHUNK, P], dt.float32, tag="t")
            q_t = qpool.tile([P, NCHUNK, P], dt.bfloat16, tag="q")
            nc.scalar.activation(t_t[:, :, :], xh[:, :, :], ACTF.Copy,
                                 bias=MAGIC, scale=rsc[:])
            nc.scalar.activation(q_t[:, :, :], t_t[:, :, :], ACTF.Copy, bias=-MAGIC)

            # transpose q chunks in bf16 (small ints, exact; bf16 transpose
            # streams at 1 cyc/row vs the slower fp8 step-2 path), then the
            # PSUM->SBUF copy casts to fp8 for the DoubleRow matmul.
            qT_ps = ps_qT.tile([P, NCHUNK, P], dt.bfloat16, tag="qT_ps")
            for c in range(NCHUNK):
                nc.tensor.transpose(qT_ps[:, c, :], q_t[:, c, :], id16[:])
            qT = qpool.tile([P, NCHUNK, P], dt.float8e4, tag="qT")
            nc.scalar.copy(qT[:, :, :], qT_ps[:, :, :])

            # M2: G = q @ tern.T (fp8, exact), epilogue scales
            o_t = opool.tile([P, NCHUNK * P], dt.float32, tag="o")
            for oh in range(2):
                g = ps_g.tile([P, 512], dt.float32, tag="g")
                for kk in range(NCHUNK // 2):
                    nc.tensor.matmul(
                        g[:], qT[:, 2 * kk : 2 * kk + 2, :],
                        ternT[:, 2 * kk : 2 * kk + 2, oh * 512 : (oh + 1) * 512],
                        start=(kk == 0), stop=(kk == NCHUNK // 2 - 1),
                        perf_mode=mybir.MatmulPerfMode.DoubleRow,
                    )
                nc.vector.scalar_tensor_tensor(
                    o_t[:, oh * 512 : (oh + 1) * 512], g[:], sc[:],
                    wsb[:, oh * 512 : (oh + 1) * 512], ALU.mult, ALU.mult,
                )
            nc.sync.dma_start(out[s0 : s0 + P, :], o_t[:])

    nc.finalize()
    return nc


def _get_nc():
    if "nc" not in _CACHE:
        _CACHE["nc"] = _build()
    return _CACHE["nc"]


def kernel(x: np.ndarray, weight: np.ndarray) -> np.ndarray:
    from concourse.bass_utils import run_bass_kernel_spmd

    assert x.shape == (8, ST * P, NCHUNK * P) and x.dtype == np.float32
    assert weight.shape == (NCHUNK * P, NCHUNK * P)

    hm = (_sylvester(7).astype(np.float32) / np.float32(32.0)).astype(np.float32)
    w32 = np.ascontiguousarray(weight, dtype=np.float32)
    nc = _get_nc()
    in_maps = [
        {"x": np.ascontiguousarray(x[i]), "w": w32, "hm": hm} for i in range(8)
    ]
    res = run_bass_kernel_spmd(nc, in_maps, core_ids=list(range(8)))
    return np.stack([res.results[i]["out"] for i in range(8)], axis=0)

